# revision 2
# baseline (speedup 1.0000x reference)
"""Trainium2 Bass kernel for DigitConvolutionalModel.

Reference computation (B = 32768):
    x: [B, 784] -> reshape [B, 28, 28]
    conv 3x3 valid with w_conv -> [B, 26, 26] -> [B, 676]
    h1 = relu(conv @ W1 + b1)    W1: [676, 100]
    h2 = relu(h1 @ W2 + b2)      W2: [100, 100]
    out = h2 @ W3 + b3           W3: [100, 10]

Strategy
--------
Pure data parallel: batch split 8 ways (4096 rows/core), weights replicated.
The conv is linear, so it is folded into W1 on the host:
    conv(x) @ W1 == x @ (M @ W1) = x @ W1e,  W1e: [784, 100]
removing the conv from the device entirely (exact up to fp rounding).

On-device layout is "transposed": features on SBUF partitions, batch on the
free dimension, so each layer's PSUM output feeds the next matmul directly
as the moving operand. The host pre-transposes x per core and lays it out
as [128, 6, B_LOC] (contraction split 784 = 6*128 + 16; the 16-row tail is
a separate [16, B_LOC] resident tile) so every x DMA uses all 128
partitions with long contiguous runs.

x and the weights are cast to fp16 on the host: fp16's 10-bit mantissa
keeps end-to-end error at the ~1e-3 level vs the fp32 reference while
halving HBM traffic and running every matmul at full PE rate. The kernel
is HBM-bandwidth bound streaming x (~6.4 MB/core).

v2 changes vs the previous kernel:
  - x streams alone on the sync HWDGE ring; all weight loads moved to the
    scalar ring so the stream starts at the earliest possible dispatch.
  - No gpsimd (SWDGE) DMAs and no ScalarE activations: outputs go out on
    the scalar HWDGE ring and the whole epilogue runs on DVE
    tensor_scalar ops, so neither the SWDGE scratch MEMSET nor the
    ACT_TABLE_LOAD appears at the head of the instruction stream.
  - Groups sized [1024 x3, 512 x2]: small first group starts the PE early,
    small last group keeps the exposed post-stream epilogue short; the
    final subtile runs its stages in 256-column halves on two PSUM banks
    so the tail chain pipelines with the PE.
"""

import numpy as np

N_CORES = 8
B = 32768
B_LOC = B // N_CORES          # 4096 rows per core
NT = 512                      # matmul moving-dim tile (PSUM bank limit)
GROUPS = [1024, 1024, 1024, 512, 512]
KC = 6                        # full 128-row contraction chunks
KT = 784 - KC * 128           # 16-row tail
H = 100                       # hidden width
O = 10                        # output width
XBUFS = 18                    # in-flight x chunk-piece DMAs
N_PS1 = 5                     # rotating layer-1 PSUM accumulator banks

_COMPILED = {}
LAST_RESULTS = None


def _build_nc():
    import concourse.mybir as mybir
    from concourse import bacc
    from concourse.tile import TileContext

    f32 = mybir.dt.float32
    f16 = mybir.dt.float16

    nc = bacc.Bacc(
        "TRN2", target_bir_lowering=False, debug=False, num_devices=N_CORES
    )
    xt = nc.dram_tensor("xt", [128, KC, B_LOC], f16, kind="ExternalInput")
    w1 = nc.dram_tensor("w1", [128, KC, H], f16, kind="ExternalInput")
    # packed [16, 100 + B_LOC]: W1e tail rows | x tail rows
    wxl = nc.dram_tensor("wxl", [KT, H + B_LOC], f16, kind="ExternalInput")
    # packed [100, 110]: W2 | W3
    w23 = nc.dram_tensor("w23", [H, H + O], f16, kind="ExternalInput")
    # packed [100, 3]: b1 | b2 | b3 (b3 on partitions 0..9)
    bb = nc.dram_tensor("bb", [H, 3], f32, kind="ExternalInput")
    ot = nc.dram_tensor("ot", [O, B_LOC], f32, kind="ExternalOutput")

    add = mybir.AluOpType.add
    amax = mybir.AluOpType.max

    with TileContext(nc) as tc:
        with (
            tc.tile_pool(name="wpool", bufs=1) as wpool,
            tc.tile_pool(name="xpool", bufs=XBUFS) as xpool,
            tc.tile_pool(name="hpool", bufs=3) as hpool,
            tc.tile_pool(name="opool", bufs=3) as opool,
            tc.tile_pool(name="ppool", bufs=1, space="PSUM") as ppool,
        ):
            # All weights on the scalar HWDGE ring; the sync ring carries
            # only the x stream so its first dispatch is the earliest
            # instruction after the preamble.
            w1_t = wpool.tile([128, KC, H], f16)
            nc.scalar.dma_start(out=w1_t, in_=w1.ap())
            wxl_t = wpool.tile([KT, H + B_LOC], f16)
            nc.scalar.dma_start(out=wxl_t, in_=wxl.ap())
            w1l_t = wxl_t[:, 0:H]
            xl_t = wxl_t[:, H : H + B_LOC]
            w23_t = wpool.tile([H, H + O], f16)
            nc.scalar.dma_start(out=w23_t, in_=w23.ap())
            bb_t = wpool.tile([H, 3], f32)
            nc.scalar.dma_start(out=bb_t, in_=bb.ap())

            w2_t = w23_t[:, 0:H]
            w3_t = w23_t[:, H : H + O]
            b1_t = bb_t[:, 0:1]
            b2_t = bb_t[:, 1:2]
            b3_t = bb_t[:O, 2:3]

            def epilogue(g0, subt, ps1s):
                # stage-major across subtiles; everything on DVE
                h1s, h2s = [], []
                for s in range(subt):
                    h1 = hpool.tile([H, NT], f16, tag="h1", bufs=4, name=f"h1_{s}")
                    nc.vector.tensor_scalar(
                        h1, ps1s[s][:H, :], b1_t, 0.0, add, amax
                    )
                    h1s.append(h1)
                for s in range(subt):
                    ps2 = ppool.tile([128, NT], f32, tag="ps2", bufs=2, name="ps2")
                    nc.tensor.matmul(
                        ps2[:H, :], lhsT=w2_t, rhs=h1s[s], start=True, stop=True
                    )
                    h2 = hpool.tile([H, NT], f16, tag="h2", bufs=4, name=f"h2_{s}")
                    nc.vector.tensor_scalar(h2, ps2[:H, :], b2_t, 0.0, add, amax)
                    h2s.append(h2)
                for s in range(subt):
                    ps3 = ppool.tile([128, NT], f32, tag="ps3", bufs=1, name="ps3")
                    nc.tensor.matmul(
                        ps3[:O, :], lhsT=w3_t, rhs=h2s[s], start=True, stop=True
                    )
                    o_t = opool.tile([O, NT], f32, tag="o_t", bufs=4, name=f"o_{s}")
                    nc.vector.tensor_scalar_add(o_t, ps3[:O, :], b3_t)
                    nc.scalar.dma_start(
                        out=ot.ap()[:, g0 + s * NT : g0 + (s + 1) * NT], in_=o_t
                    )

            def epilogue_last(g0, ps1h):
                # final 512 columns: run every stage in 256-column halves on
                # the two accumulator banks so DVE and PE pipeline and the
                # exposed post-stream chain is short
                NH = NT // 2
                h1 = hpool.tile([H, NT], f16, tag="h1", bufs=4, name="h1_l")
                h2 = hpool.tile([H, NT], f16, tag="h2", bufs=4, name="h2_l")
                o_t = opool.tile([O, NT], f32, tag="o_t", bufs=4, name="o_l")
                ps2 = ppool.tile([128, NT], f32, tag="ps2", bufs=2, name="ps2")
                ps3 = ppool.tile([128, NT], f32, tag="ps3", bufs=1, name="ps3")
                for hh in range(2):
                    cs = slice(hh * NH, (hh + 1) * NH)
                    nc.vector.tensor_scalar(
                        h1[:, cs], ps1h[hh][:H, :], b1_t, 0.0, add, amax
                    )
                    nc.tensor.matmul(
                        ps2[:H, cs], lhsT=w2_t, rhs=h1[:, cs],
                        start=True, stop=True,
                    )
                    nc.vector.tensor_scalar(
                        h2[:, cs], ps2[:H, cs], b2_t, 0.0, add, amax
                    )
                    nc.tensor.matmul(
                        ps3[:O, cs], lhsT=w3_t, rhs=h2[:, cs],
                        start=True, stop=True,
                    )
                    nc.vector.tensor_scalar_add(o_t[:, cs], ps3[:O, cs], b3_t)
                    nc.scalar.dma_start(
                        out=ot.ap()[:, g0 + hh * NH : g0 + (hh + 1) * NH],
                        in_=o_t[:, cs],
                    )

            pending = None  # (g0, subt, ps1s)
            ps1_rot = 0
            g0 = 0
            n_groups = len(GROUPS)
            for g, ntd in enumerate(GROUPS):
                last = g == n_groups - 1
                gs = slice(g0, g0 + ntd)
                subt = ntd // NT
                xc = []
                for c in range(KC):
                    x_c = xpool.tile([128, ntd], f16, tag="xc", name=f"xc{c}")
                    nc.sync.dma_start(out=x_c, in_=xt.ap()[:, c, gs])
                    xc.append(x_c)

                if last:
                    # two half-width accumulators for the final 512 columns
                    ps1h = [
                        ppool.tile(
                            [128, NT // 2], f32,
                            tag=f"ps1_{(ps1_rot + s) % N_PS1}",
                            bufs=1, name=f"ps1h_{s}",
                        )
                        for s in range(2)
                    ]
                    ps1_rot += 2
                    NH = NT // 2
                    for c in range(KC):
                        for s in range(2):
                            nc.tensor.matmul(
                                ps1h[s][:H, :],
                                lhsT=w1_t[:, c, :],
                                rhs=xc[c][:, s * NH : (s + 1) * NH],
                                start=(c == 0),
                                stop=(c == KC - 1),
                            )
                        if c == 2:
                            for s in range(2):
                                nc.tensor.matmul(
                                    ps1h[s][:H, :],
                                    lhsT=w1l_t,
                                    rhs=xl_t[:, g0 + s * NH : g0 + (s + 1) * NH],
                                    start=False,
                                    stop=False,
                                )
                        if c == 0 and pending is not None:
                            epilogue(*pending)
                            pending = None
                    epilogue_last(g0, ps1h)
                else:
                    ps1s = [
                        ppool.tile(
                            [128, NT], f32,
                            tag=f"ps1_{(ps1_rot + s) % N_PS1}",
                            bufs=1, name=f"ps1_{s}",
                        )
                        for s in range(subt)
                    ]
                    ps1_rot += subt
                    for c in range(KC):
                        for s in range(subt):
                            nc.tensor.matmul(
                                ps1s[s][:H, :],
                                lhsT=w1_t[:, c, :],
                                rhs=xc[c][:, s * NT : (s + 1) * NT],
                                start=(c == 0),
                                stop=(c == KC - 1),
                            )
                        if c == 2:
                            for s in range(subt):
                                nc.tensor.matmul(
                                    ps1s[s][:H, :],
                                    lhsT=w1l_t,
                                    rhs=xl_t[:, g0 + s * NT : g0 + (s + 1) * NT],
                                    start=False,
                                    stop=False,
                                )
                        if c == 0 and pending is not None:
                            epilogue(*pending)
                            pending = None
                    pending = (g0, subt, ps1s)
                g0 += ntd

    nc.finalize()
    return nc


def _fold_conv_into_w1(w_conv, W1):
    """W1e[784, 100] such that x @ W1e == conv3x3(x) @ W1 (exact linear fold)."""
    W1e = np.zeros((28, 28, H), np.float64)
    W1r = W1.astype(np.float64).reshape(26, 26, H)
    wc = w_conv.astype(np.float64)
    for di in range(3):
        for dj in range(3):
            W1e[di : di + 26, dj : dj + 26, :] += wc[di, dj] * W1r
    return W1e.reshape(784, H).astype(np.float32)


def kernel(x, w_conv, W1, b1, W2, b2, W3, b3):
    from concourse.bass_utils import run_bass_kernel_spmd

    global LAST_RESULTS

    x = np.asarray(x, np.float32)
    W1e = _fold_conv_into_w1(np.asarray(w_conv), np.asarray(W1))
    # [784, 100]: rows 0..767 -> [128, KC, 100]; rows 768..783 -> [16, 100]
    w1_dev = np.ascontiguousarray(
        W1e[: KC * 128].reshape(KC, 128, H).transpose(1, 0, 2)
    ).astype(np.float16)
    w1l_dev = W1e[KC * 128 :].astype(np.float16)      # [16, 100]
    w23_dev = np.zeros((H, H + O), np.float16)
    w23_dev[:, 0:H] = np.asarray(W2, np.float32).astype(np.float16)
    w23_dev[:, H : H + O] = np.asarray(W3, np.float32).astype(np.float16)
    bb_dev = np.zeros((H, 3), np.float32)
    bb_dev[:, 0] = np.asarray(b1, np.float32)
    bb_dev[:, 1] = np.asarray(b2, np.float32)
    bb_dev[:O, 2] = np.asarray(b3, np.float32)

    in_maps = []
    for c in range(N_CORES):
        xs = x[c * B_LOC : (c + 1) * B_LOC]          # [B_LOC, 784]
        xT = xs.T.astype(np.float16)                  # [784, B_LOC] fp16
        # main: [128, KC, B_LOC], element [p, k, n] = xT[k*128 + p, n]
        xmain = np.ascontiguousarray(
            xT[: KC * 128].reshape(KC, 128, B_LOC).transpose(1, 0, 2)
        )
        wxl_dev = np.concatenate([w1l_dev, xT[KC * 128 :]], axis=1)
        in_maps.append(
            {
                "xt": xmain,
                "wxl": np.ascontiguousarray(wxl_dev),
                "w1": w1_dev,
                "w23": w23_dev,
                "bb": bb_dev,
            }
        )

    if "nc" not in _COMPILED:
        _COMPILED["nc"] = _build_nc()
    nc = _COMPILED["nc"]

    res = run_bass_kernel_spmd(nc, in_maps, core_ids=list(range(N_CORES)))
    LAST_RESULTS = res

    out = np.empty((B, O), np.float32)
    for c in range(N_CORES):
        out[c * B_LOC : (c + 1) * B_LOC] = res.results[c]["ot"].T
    return out
